# revision 13
# baseline (speedup 1.0000x reference)
"""Trainium2 Bass kernel for nn_Attention_62010737820049.

Transformer-XL-style relative-position attention block + LN + FFN,
data-parallel over batch across 8 NeuronCores (4 batches per core, no
collectives).

v4 design — algebraic refactor on top of v2:
- Attention logits computed as (x @ Wqke) @ hf^T with Wqke = Wke^T Wq
  folded on host: kills the separate q and k projections (att path is
  192 matmuls/batch instead of 320).
- Rel-pos matrix computed as x @ (kr Wq)^T; the u/bq part of its skew
  plus v@kr^T are folded into one host-side additive table (vkadj).
- Value/MLP path computed as o2 = (att^T @ hf_nat) @ Wvm^T with
  Wvm = Wmlp Wv folded on host: kills the val projection (128 matmuls
  instead of 192). bv enters through bmlp via softmax-rows-sum-to-1.
- x/h uploaded bf16; hf^T built by xbar DMA-transpose; hf_nat is a
  plain load of the same bytes.
- Minimal-mask softmax: one lower-triangular 128-col diagonal block;
  fully-masked cols memset to exp(0)=1; 1/sqrt(D) via activation scale.
- The (q+u)kr scratch matrix round-trips DRAM in bf16 with the
  circulant shift realized as a stride-(W-1) strided re-read.
- Weights stream in 2MB batched DMAs; PSUM evacuations alternate
  between ScalarE and VectorE.
- k-bias (bke) support compiles in a per-row logit-offset path only
  when bke != 0 (it is zero for the reference inputs).
"""

import os
import sys

sys.path.insert(0, "/opt/trn_rl_repo")

import numpy as np
import ml_dtypes

B, C, MEM, D = 32, 512, 512, 1024
W = C + MEM           # 1024
FF = 4 * D            # 4096
P = 128
NCORES = 8
BPC = int(os.environ.get("KERNEL_BPC", str(B // NCORES)))  # batches per core
CH_D, CH_C, CH_W, CH_F = D // P, C // P, W // P, FF // P   # 8, 4, 8, 32
EPS = 1e-5
ISQ = 1.0 / 32.0      # 1/sqrt(D)

_cached = {}


def _emit(nc, tc, tn, has_bke):
    import concourse.bass as bass
    import concourse.mybir as mybir
    from concourse.masks import make_identity

    f32 = mybir.dt.float32
    bf16 = mybir.dt.bfloat16
    AF = mybir.ActivationFunctionType
    OP = mybir.AluOpType

    def vtt(out, a, b, op):
        return nc.vector.tensor_tensor(out=out, in0=a, in1=b, op=op)

    xs, hs, outs = tn["x"], tn["h"], tn["out"]

    alt_state = [0]

    def evac(dst, src, bias=None):
        """PSUM -> SBUF evacuation alternating between ACT and DVE."""
        alt_state[0] ^= 1
        if bias is None:
            if alt_state[0]:
                nc.scalar.copy(dst, src)
            else:
                nc.vector.tensor_copy(dst, src)
        else:
            if alt_state[0]:
                nc.scalar.activation(dst, src, AF.Identity, bias=bias)
            else:
                nc.vector.tensor_scalar(
                    out=dst, in0=src, scalar1=bias, scalar2=None, op0=OP.add
                )

    with (
        tc.tile_pool(name="constp", bufs=1) as constp,
        tc.tile_pool(name="wpool", bufs=int(os.environ.get("KERNEL_WBUFS", "3"))) as wpool,
        tc.tile_pool(name="hv", bufs=16) as hv,
        tc.tile_pool(name="scp", bufs=24) as scp,
        tc.tile_pool(name="ftp", bufs=1) as ftp,
        tc.tile_pool(name="z3p", bufs=2) as z3p,
        tc.tile_pool(name="sx4", bufs=5) as sx4,
        tc.tile_pool(name="e2", bufs=10) as e2,
        tc.tile_pool(name="stp", bufs=24) as stp,
        tc.tile_pool(name="psp", bufs=8, space="PSUM") as psp,
        tc.tile_pool(name="pdram", bufs=2, space="DRAM") as pdram,
    ):
        identb = constp.tile([P, P], bf16, name="identb")
        make_identity(nc, identb[:])
        # lower-triangular (incl. diagonal) 0/1 mask for the band edge
        tri = constp.tile([P, P], f32, name="tri")
        nc.gpsimd.memset(tri[:], 1.0)
        nc.gpsimd.affine_select(
            out=tri[:], in_=tri[:], compare_op=OP.is_ge, fill=0.0,
            base=0, pattern=[[-1, P]], channel_multiplier=1,
        )
        bf1_sb = constp.tile([P, CH_F], f32, name="bf1_sb")
        nc.sync.dma_start(out=bf1_sb[:], in_=tn["bf1p"].rearrange("(k p) -> p k", p=P))
        ones_sb = constp.tile([1, P], bf16, name="ones_sb")
        nc.sync.dma_start(out=ones_sb[:], in_=tn["ones_row"][:, :])
        bmr = constp.tile([1, D], bf16, name="bmr")
        nc.sync.dma_start(out=bmr[:], in_=tn["bmlp_row"][:, :])
        bf2r = constp.tile([1, D], bf16, name="bf2r")
        nc.sync.dma_start(out=bf2r[:], in_=tn["bf2_row"][:, :])
        u_ke = []
        for dc in range(CH_D):
            ut = constp.tile([P, C], bf16, name=f"uke{dc}")
            nc.sync.dma_start(out=ut[:], in_=tn["u_ke"][dc * P:(dc + 1) * P, :])
            u_ke.append(ut)
        if has_bke:
            wqbke = constp.tile([P, CH_D], f32, name="wqbke")
            nc.sync.dma_start(
                out=wqbke[:], in_=tn["wqbke"].rearrange("(k p) -> p k", p=P))
            ubke = constp.tile([P, CH_C], f32, name="ubke")
            nc.sync.dma_start(
                out=ubke[:], in_=tn["ubke"].rearrange("(k p) -> p k", p=P))

        def _one_pass():
            for bi in range(BPC):
                # ---- hf_T [d, w] = (h | x)^T via xbar DMA; hf_nat plain ----
                hf = []
                for dc in range(CH_D):
                    t = hv.tile([P, W], bf16, name=f"hf{dc}", tag="hv")
                    nc.sync.dma_start(
                        out=t[:, 0:MEM], in_=hs[bi, :, dc * P:(dc + 1) * P],
                        transpose=True,
                    )
                    nc.sync.dma_start(
                        out=t[:, MEM:W], in_=xs[bi, :, dc * P:(dc + 1) * P],
                        transpose=True,
                    )
                    hf.append(t)
                hfn = []
                for wc in range(CH_W):
                    t = hv.tile([P, D], bf16, name=f"hfn{wc}", tag="hv")
                    src = hs if wc < CH_C else xs
                    ro = wc * P if wc < CH_C else (wc - CH_C) * P
                    nc.sync.dma_start(out=t[:], in_=src[bi, ro:ro + P, :])
                    hfn.append(t)

                # ---- S: sT = Wqke^T x^T + u_ke  [d_in, c] ----
                wqke = wpool.tile([P, CH_D, D], bf16, name="wqke", tag="w")
                nc.sync.dma_start(out=wqke[:], in_=tn["wqke_r"][:, :, :])
                sT = []
                for do in range(CH_D):
                    sps = psp.tile([P, C], f32, name="sps", tag="ps")
                    for ki in range(CH_D):
                        nc.tensor.matmul(
                            sps[:],
                            wqke[:, ki, do * P:(do + 1) * P],
                            hf[ki][:, MEM:W],
                            start=(ki == 0), stop=(ki == CH_D - 1),
                        )
                    st = scp.tile([P, C], bf16, name=f"sT{do}", tag="sc")
                    vtt(st[:], sps[:], u_ke[do][:], OP.add)
                    sT.append(st)
                dume = stp.tile([P, 1], f32, name="dume", tag="st")
                nc.scalar.activation(dume[:], sT[0][:, 0:1], AF.Exp)

                # ---- E: Pm = x @ krq^T -> DRAM scratch (bf16) ----
                krq = wpool.tile([P, CH_D, W], bf16, name="krq", tag="w")
                nc.sync.dma_start(out=krq[:], in_=tn["krq_r"][:, :, :])
                pd = pdram.tile([C * W], bf16, name="pd", tag="pd")
                pd2 = pd.rearrange("(c w) -> c w", w=W)
                for ci in range(CH_C):
                    pps = [psp.tile([P, C], f32, name=f"pps{hh}", tag="ps")
                           for hh in range(2)]
                    for ki in range(CH_D):
                        for hh in range(2):
                            nc.tensor.matmul(
                                pps[hh][:],
                                hf[ki][:, MEM + ci * P:MEM + (ci + 1) * P],
                                krq[:, ki, hh * 512:(hh + 1) * 512],
                                start=(ki == 0), stop=(ki == CH_D - 1),
                            )
                    for hh in range(2):
                        psb = e2.tile([P, C], bf16, name="psb", tag="e2")
                        evac(psb[:], pps[hh][:])
                        nc.sync.dma_start(
                            out=pd2[ci * P:(ci + 1) * P, hh * 512:(hh + 1) * 512],
                            in_=psb[:],
                        )

                # ---- delta row-offset (only when bke != 0) ----
                deltas = []
                if has_bke:
                    for ci in range(CH_C):
                        dps = psp.tile([P, 1], f32, name="dps", tag="ps")
                        for ki in range(CH_D):
                            nc.tensor.matmul(
                                dps[:],
                                hf[ki][:, MEM + ci * P:MEM + (ci + 1) * P],
                                wqbke[:, ki:ki + 1],
                                start=(ki == 0), stop=(ki == CH_D - 1),
                            )
                        dcol = stp.tile([P, 1], f32, name="dcol", tag="st")
                        vtt(dcol[:], dps[:], ubke[:, ci:ci + 1], OP.add)
                        deltas.append(dcol)

                # ---- F+O fused per row-block: logits, unnormalized softmax,
                #      transpose, att@hf; 1/z folded into the oh evacuation ----
                es, rzs = [], []
                for ci in range(CH_C):
                    active = ci * P + 640          # cols >= active fully masked
                    up = active - 512
                    pskew = e2.tile([P, W], bf16, name="pskew", tag="e2")
                    skew_ap = bass.AP(
                        tensor=pd.tensor,
                        offset=pd.offset + (W - 1) * P * ci + C - 1,
                        ap=[[W - 1, P], [1, W]],
                    )
                    nc.sync.dma_start(out=pskew[:], in_=skew_ap)
                    vkt = e2.tile([P, W], bf16, name="vkt", tag="e2")
                    nc.sync.dma_start(out=vkt[:], in_=tn["vkadj"][ci * P:(ci + 1) * P, :])
                    pre = sx4.tile([P, W], f32, name="pre", tag="sx")
                    vtt(pre[:, 0:active], pskew[:, 0:active], vkt[:, 0:active], OP.add)
                    aps0 = psp.tile([P, C], f32, name="aps0", tag="ps")
                    aps1 = psp.tile([P, up], f32, name="aps1", tag="ps")
                    for ki in range(CH_D):
                        nc.tensor.matmul(
                            aps0[:],
                            sT[ki][:, ci * P:(ci + 1) * P],
                            hf[ki][:, 0:512],
                            start=(ki == 0), stop=(ki == CH_D - 1),
                        )
                        nc.tensor.matmul(
                            aps1[:],
                            sT[ki][:, ci * P:(ci + 1) * P],
                            hf[ki][:, 512:512 + up],
                            start=(ki == 0), stop=(ki == CH_D - 1),
                        )
                    t = sx4.tile([P, W], f32, name="t", tag="sx")
                    vtt(t[:, 0:512], aps0[:], pre[:, 0:512], OP.add)
                    vtt(t[:, 512:active], aps1[:], pre[:, 512:active], OP.add)
                    if has_bke:
                        nc.vector.tensor_scalar(
                            out=t[:, 0:active], in0=t[:, 0:active],
                            scalar1=deltas[ci][:], scalar2=None, op0=OP.add,
                        )
                    vtt(t[:, active - P:active], t[:, active - P:active],
                        tri[:], OP.mult)
                    e = e2.tile([P, W], bf16, name=f"e{ci}", tag="e2")
                    zrow = stp.tile([P, 1], f32, name="zrow", tag="st")
                    if active < W:
                        nc.vector.memset(e[:, active:W], 1.0)
                    nc.scalar.activation(e[:, 0:active], t[:, 0:active], AF.Exp,
                                         scale=ISQ, accum_out=zrow[:])
                    if active < W:
                        nc.vector.tensor_scalar_add(zrow[:], zrow[:], float(W - active))
                    rz = stp.tile([P, 1], f32, name="rz", tag="st")
                    nc.vector.reciprocal(rz[:], zrow[:])
                    rzs.append(rz)
                    es.append(e)
                attT = []
                for wc in range(CH_W):
                    tp = psp.tile([P, C], bf16, name="tp2", tag="ps")
                    for ci in range(CH_C):
                        nc.tensor.transpose(
                            tp[:, ci * P:(ci + 1) * P],
                            es[ci][:, wc * P:(wc + 1) * P],
                            identb[:],
                        )
                    at = scp.tile([P, C], bf16, name=f"attT{wc}", tag="sc")
                    evac(at[:], tp[:])
                    attT.append(at)

                # ---- O: oh = att @ hf  [c, d]; 1/z applied at evacuation ----
                ohps = [
                    [psp.tile([P, C], f32, name=f"oh{ci}{hh}", tag="ps") for hh in range(2)]
                    for ci in range(CH_C)
                ]
                for wc in range(CH_W):
                    for ci in range(CH_C):
                        for hh in range(2):
                            nc.tensor.matmul(
                                ohps[ci][hh][:],
                                attT[wc][:, ci * P:(ci + 1) * P],
                                hfn[wc][:, hh * 512:(hh + 1) * 512],
                                start=(wc == 0), stop=(wc == CH_W - 1),
                            )
                ohs = []
                for ci in range(CH_C):
                    oh = e2.tile([P, D], bf16, name="ohsb", tag="e2")
                    for hh in range(2):
                        alt_state[0] ^= 1
                        if alt_state[0]:
                            nc.scalar.mul(oh[:, hh * 512:(hh + 1) * 512],
                                          ohps[ci][hh][:], rzs[ci][:])
                        else:
                            nc.vector.tensor_scalar_mul(
                                oh[:, hh * 512:(hh + 1) * 512], ohps[ci][hh][:],
                                rzs[ci][:])
                    ohs.append(oh)
                oT = []
                for dc in range(CH_D):
                    tp = psp.tile([P, C], bf16, name="tp4", tag="ps")
                    for ci in range(CH_C):
                        nc.tensor.transpose(
                            tp[:, ci * P:(ci + 1) * P],
                            ohs[ci][:, dc * P:(dc + 1) * P],
                            identb[:],
                        )
                    ot = scp.tile([P, C], bf16, name=f"oT{dc}", tag="sc")
                    evac(ot[:], tp[:])
                    oT.append(ot)
                dums = stp.tile([P, 1], f32, name="dums", tag="st")
                nc.scalar.activation(dums[:], tri[:, 0:1], AF.Sqrt)

                # ---- H: o2 = oh @ Wvm^T + bmlp_adj ; LN -> z (bf16) ; zT ----
                wvm = wpool.tile([P, CH_D, D], bf16, name="wvm", tag="w")
                nc.sync.dma_start(out=wvm[:], in_=tn["wvm_r"][:, :, :])
                zs = []
                for ci in range(CH_C):
                    o2 = sx4.tile([P, D], f32, name="o2", tag="sx")
                    o2ps = [psp.tile([P, C], f32, name=f"o2ps{hh}", tag="ps")
                            for hh in range(2)]
                    for ki in range(CH_D):
                        for hh in range(2):
                            nc.tensor.matmul(
                                o2ps[hh][:],
                                oT[ki][:, ci * P:(ci + 1) * P],
                                wvm[:, ki, hh * 512:(hh + 1) * 512],
                                start=(ki == 0), stop=False,
                            )
                    s = []
                    for hh in range(2):
                        nc.tensor.matmul(
                            o2ps[hh][:], ones_sb[:1, :P], bmr[:1, hh * 512:(hh + 1) * 512],
                            start=False, stop=True,
                        )
                        sh = stp.tile([P, 1], f32, name="sh", tag="st")
                        nc.scalar.activation(
                            o2[:, hh * 512:(hh + 1) * 512], o2ps[hh][:], AF.Identity,
                            bias=0.0, accum_out=sh[:],
                        )
                        s.append(sh)
                    sq = sx4.tile([P, D], f32, name="sq", tag="sx")
                    ss0 = stp.tile([P, 1], f32, name="ss0", tag="st")
                    ss1 = stp.tile([P, 1], f32, name="ss1", tag="st")
                    nc.scalar.activation(sq[:, 0:512], o2[:, 0:512], AF.Square,
                                         bias=0.0, accum_out=ss0[:])
                    nc.scalar.activation(sq[:, 512:1024], o2[:, 512:1024], AF.Square,
                                         bias=0.0, accum_out=ss1[:])
                    ss = stp.tile([P, 1], f32, name="ss", tag="st")
                    vtt(ss[:], ss0[:], ss1[:], OP.add)
                    mu = stp.tile([P, 1], f32, name="mu", tag="st")
                    vtt(mu[:], s[0][:], s[1][:], OP.add)
                    nc.vector.tensor_scalar_mul(mu[:], mu[:], 1.0 / D)
                    ex2 = stp.tile([P, 1], f32, name="ex2", tag="st")
                    nc.vector.tensor_scalar_mul(ex2[:], ss[:], 1.0 / D)
                    var = stp.tile([P, 1], f32, name="var", tag="st")
                    vtt(var[:], mu[:], mu[:], OP.mult)
                    vtt(var[:], ex2[:], var[:], OP.subtract)
                    nc.vector.tensor_scalar_add(var[:], var[:], EPS)
                    sd = stp.tile([P, 1], f32, name="sd", tag="st")
                    nc.scalar.activation(sd[:], var[:], AF.Sqrt, bias=0.0)
                    rstd = stp.tile([P, 1], f32, name="rstd", tag="st")
                    nc.vector.reciprocal(rstd[:], sd[:])
                    zc = e2.tile([P, D], bf16, name="z", tag="e2")
                    nc.vector.tensor_scalar(
                        out=zc[:], in0=o2[:], scalar1=mu[:], scalar2=rstd[:],
                        op0=OP.subtract, op1=OP.mult,
                    )
                    zs.append(zc)
                zT3 = z3p.tile([P, CH_D, C], bf16, name="zT3", tag="z3")
                for dc in range(CH_D):
                    tp = psp.tile([P, C], bf16, name="tp3", tag="ps")
                    for ci in range(CH_C):
                        nc.tensor.transpose(
                            tp[:, ci * P:(ci + 1) * P],
                            zs[ci][:, dc * P:(dc + 1) * P],
                            identb[:],
                        )
                    evac(zT3[:, dc, :], tp[:])

                # ---- I: f_T = relu(Wf1g z_T + bf1') ----
                fT3 = ftp.tile([P, CH_F, C], bf16, name="fT3", tag="ft")
                for jj in range(4):
                    w1c = wpool.tile([P, CH_D, 1024], bf16, name=f"w1c{jj}", tag="w")
                    nc.sync.dma_start(
                        out=w1c[:], in_=tn["w1_r"][:, :, jj * 1024:(jj + 1) * 1024]
                    )
                    for j in range(8):
                        jc = jj * 8 + j
                        fps = psp.tile([P, C], f32, name="fps", tag="ps")
                        for dc in range(CH_D):
                            nc.tensor.matmul(
                                fps[:], w1c[:, dc, j * P:(j + 1) * P], zT3[:, dc, :],
                                start=(dc == 0), stop=(dc == CH_D - 1),
                            )
                        alt_state[0] ^= 1
                        if alt_state[0]:
                            nc.scalar.activation(fT3[:, jc, :], fps[:], AF.Relu,
                                                 bias=bf1_sb[:, jc:jc + 1])
                        else:
                            nc.vector.tensor_scalar(
                                out=fT3[:, jc, :], in0=fps[:],
                                scalar1=bf1_sb[:, jc:jc + 1],
                                scalar2=0.0, op0=OP.add, op1=OP.max,
                            )

                # ---- J: out = f @ Wf2^T + bf2 ----
                outps = [
                    [psp.tile([P, C], f32, name=f"op{ci}{hh}", tag="ps") for hh in range(2)]
                    for ci in range(CH_C)
                ]
                for jj in range(4):
                    w2c = wpool.tile([P, 8, D], bf16, name=f"w2c{jj}", tag="w")
                    nc.sync.dma_start(
                        out=w2c[:], in_=tn["w2_r"][:, jj * 8:(jj + 1) * 8, :]
                    )
                    for j in range(8):
                        jc = jj * 8 + j
                        for ci in range(CH_C):
                            for hh in range(2):
                                nc.tensor.matmul(
                                    outps[ci][hh][:],
                                    fT3[:, jc, ci * P:(ci + 1) * P],
                                    w2c[:, j, hh * 512:(hh + 1) * 512],
                                    start=(jc == 0), stop=False,
                                )
                for ci in range(CH_C):
                    ob = sx4.tile([P, D], f32, name="ob", tag="sx")
                    for hh in range(2):
                        nc.tensor.matmul(
                            outps[ci][hh][:], ones_sb[:1, :P],
                            bf2r[:1, hh * 512:(hh + 1) * 512],
                            start=False, stop=True,
                        )
                        evac(ob[:, hh * 512:(hh + 1) * 512], outps[ci][hh][:])
                    nc.sync.dma_start(out=outs[bi, ci * P:(ci + 1) * P, :], in_=ob[:])

        LOOP_R = int(os.environ.get("KERNEL_LOOP", "0"))
        if LOOP_R > 1:
            with tc.For_i(0, LOOP_R, 1):
                _one_pass()
        else:
            _one_pass()


def _build(has_bke=False):
    key = ("nc", has_bke)
    if key in _cached:
        return _cached[key]
    import concourse.mybir as mybir
    import concourse.tile as tile
    from concourse import bacc

    f32 = mybir.dt.float32
    bf16 = mybir.dt.bfloat16
    nc = bacc.Bacc("TRN2", target_bir_lowering=False, debug=False,
                   num_devices=NCORES)
    tn = {}
    tn["x"] = nc.dram_tensor("x", [BPC, C, D], bf16, kind="ExternalInput")
    tn["h"] = nc.dram_tensor("h", [BPC, MEM, D], bf16, kind="ExternalInput")
    for nm, shp in [
        ("wqke_r", [P, CH_D, D]), ("wvm_r", [P, CH_D, D]), ("krq_r", [P, CH_D, W]),
        ("w1_r", [P, CH_D, FF]), ("w2_r", [P, CH_F, D]),
        ("u_ke", [D, C]), ("vkadj", [C, W]),
        ("ones_row", [1, P]), ("bmlp_row", [1, D]), ("bf2_row", [1, D]),
    ]:
        tn[nm] = nc.dram_tensor(nm, shp, bf16, kind="ExternalInput")
    for nm, shp in [("bf1p", [FF])]:
        tn[nm] = nc.dram_tensor(nm, shp, f32, kind="ExternalInput")
    if has_bke:
        tn["wqbke"] = nc.dram_tensor("wqbke", [D], f32, kind="ExternalInput")
        tn["ubke"] = nc.dram_tensor("ubke", [C], f32, kind="ExternalInput")
    tn["out"] = nc.dram_tensor("out", [BPC, C, D], f32, kind="ExternalOutput")

    with tile.TileContext(nc) as tc:
        _emit(nc, tc, tn, has_bke)
    nc.compile()
    _cached[key] = nc
    return nc


def _circulant_idx(height, width, shift):
    i = np.roll(np.arange(width), shift)[::-1]
    i2 = np.concatenate([i, i])
    win = np.lib.stride_tricks.sliding_window_view(i2, width)
    win = win[:, ::-1]
    return np.ascontiguousarray(win[:height])


def _host_consts(inputs):
    f = np.float32
    bf = ml_dtypes.bfloat16
    Wq, bq = inputs["Wq"].astype(f), inputs["bq"].astype(f)
    Wke, bke = inputs["Wke"].astype(f), inputs["bke"].astype(f)
    Wkr, bkr = inputs["Wkr"].astype(f), inputs["bkr"].astype(f)
    Wv, bv = inputs["Wv"].astype(f), inputs["bv"].astype(f)
    Wmlp, bmlp = inputs["Wmlp"].astype(f), inputs["bmlp"].astype(f)
    gamma, beta = inputs["gamma"].astype(f), inputs["beta"].astype(f)
    Wf1, bf1 = inputs["Wf1"].astype(f), inputs["bf1"].astype(f)
    Wf2, bf2 = inputs["Wf2"].astype(f), inputs["bf2"].astype(f)
    u, v, rr = inputs["u"].astype(f), inputs["v"].astype(f), inputs["r"].astype(f)

    kr = rr @ Wkr.T + bkr                      # [W, D]
    u_b = u + bq[None, :]                      # bq rides along with u
    vkr = v @ kr.T                             # [C, W]
    ukr = u_b @ kr.T                           # [C, W]
    idx = _circulant_idx(C, W, -C + 1)
    skew_ukr = np.take_along_axis(ukr, idx, axis=1)
    vkadj = vkr - skew_ukr                     # unscaled; mask handled on-chip

    Wqke = Wke.T @ Wq                          # s^T = Wqke^T x^T (+ (u Wke)^T)
    krq = kr @ Wq                              # [W, D]: x @ krq^T = (x Wq^T) kr^T
    Wvm = Wmlp @ Wv                            # o2 = oh @ Wvm^T

    def pkd(Wt, width):                        # [D, width] -> [P, CH_D, width]
        return np.ascontiguousarray(
            Wt.reshape(CH_D, P, width).transpose(1, 0, 2)
        ).astype(bf)

    cn = {
        "wqke_r": pkd(np.ascontiguousarray(Wqke.T), D),
        "wvm_r": pkd(np.ascontiguousarray(Wvm.T), D),
        "krq_r": pkd(np.ascontiguousarray(krq.T), W),
        "w1_r": pkd(np.ascontiguousarray((Wf1 * gamma[None, :]).T), FF),
        "w2_r": np.ascontiguousarray(
            Wf2.T.reshape(CH_F, P, D).transpose(1, 0, 2)
        ).astype(bf),
        "u_ke": np.ascontiguousarray((u_b @ Wke).T).astype(bf),
        "vkadj": vkadj.astype(bf),
        "bf1p": (bf1 + Wf1 @ beta).astype(f),
        "ones_row": np.ones((1, P), bf),
        "bmlp_row": (bmlp + Wmlp @ bv).reshape(1, D).astype(bf),
        "bf2_row": bf2.reshape(1, D).astype(bf),
    }
    has_bke = bool(np.any(bke != 0.0))
    if has_bke:
        cn["wqbke"] = (Wq.T @ bke).astype(f)
        cn["ubke"] = (u_b @ bke).astype(f)
    return cn, has_bke


def kernel(**inputs):
    from concourse.bass_utils import run_bass_kernel_spmd

    cn, has_bke = _host_consts(inputs)
    nc = _build(has_bke)
    bf = ml_dtypes.bfloat16
    x = np.ascontiguousarray(inputs["x"]).astype(bf)
    h = np.ascontiguousarray(inputs["h"]).astype(bf)
    in_maps = []
    for i in range(NCORES):
        m = dict(cn)
        m["x"] = np.ascontiguousarray(x[i * BPC:(i + 1) * BPC])
        m["h"] = np.ascontiguousarray(h[i * BPC:(i + 1) * BPC])
        in_maps.append(m)
    res = run_bass_kernel_spmd(nc, in_maps, list(range(NCORES)))
    out = np.concatenate([res.results[i]["out"] for i in range(NCORES)], axis=0)
    return out.astype(np.float32)


# revision 26
# speedup vs baseline: 1.3299x; 1.3299x over previous
"""Trainium2 Bass kernel for nn_Attention_62010737820049.

Transformer-XL-style relative-position attention block + LN + FFN,
data-parallel over batch across 8 NeuronCores (4 batches per core, no
collectives).

v4 design — algebraic refactor on top of v2:
- Attention logits computed as (x @ Wqke) @ hf^T with Wqke = Wke^T Wq
  folded on host: kills the separate q and k projections (att path is
  192 matmuls/batch instead of 320).
- Rel-pos matrix computed as x @ (kr Wq)^T; the u/bq part of its skew
  plus v@kr^T are folded into one host-side additive table (vkadj).
- Value/MLP path computed as o2 = (att^T @ hf_nat) @ Wvm^T with
  Wvm = Wmlp Wv folded on host: kills the val projection (128 matmuls
  instead of 192). bv enters through bmlp via softmax-rows-sum-to-1.
- x/h uploaded bf16; hf^T built by xbar DMA-transpose; hf_nat is a
  plain load of the same bytes.
- Minimal-mask softmax: one lower-triangular 128-col diagonal block;
  fully-masked cols memset to exp(0)=1; 1/sqrt(D) via activation scale.
- The (q+u)kr scratch matrix round-trips DRAM in bf16 with the
  circulant shift realized as a stride-(W-1) strided re-read.
- Weights stream in 2MB batched DMAs; PSUM evacuations alternate
  between ScalarE and VectorE.
- k-bias (bke) support compiles in a per-row logit-offset path only
  when bke != 0 (it is zero for the reference inputs).
"""

import os
import sys

sys.path.insert(0, "/opt/trn_rl_repo")

import numpy as np
import ml_dtypes

B, C, MEM, D = 32, 512, 512, 1024
W = C + MEM           # 1024
FF = 4 * D            # 4096
P = 128
NCORES = 8
BPC = int(os.environ.get("KERNEL_BPC", str(B // NCORES)))  # batches per core
CH_D, CH_C, CH_W, CH_F = D // P, C // P, W // P, FF // P   # 8, 4, 8, 32
EPS = 1e-5
ISQ = 1.0 / 32.0      # 1/sqrt(D)

_cached = {}


def _emit(nc, tc, tn, has_bke):
    import concourse.bass as bass
    import concourse.mybir as mybir
    from concourse.masks import make_identity

    f32 = mybir.dt.float32
    bf16 = mybir.dt.bfloat16
    AF = mybir.ActivationFunctionType
    OP = mybir.AluOpType

    def vtt(out, a, b, op):
        return nc.vector.tensor_tensor(out=out, in0=a, in1=b, op=op)

    xs, hs, outs = tn["x"], tn["h"], tn["out"]

    alt_state = [0]

    def evac(dst, src, bias=None):
        """PSUM -> SBUF evacuation alternating between ACT and DVE."""
        alt_state[0] ^= 1
        if bias is None:
            if alt_state[0]:
                nc.scalar.copy(dst, src)
            else:
                nc.vector.tensor_copy(dst, src)
        else:
            if alt_state[0]:
                nc.scalar.activation(dst, src, AF.Identity, bias=bias)
            else:
                nc.vector.tensor_scalar(
                    out=dst, in0=src, scalar1=bias, scalar2=None, op0=OP.add
                )

    with (
        tc.tile_pool(name="constp", bufs=1) as constp,
        tc.tile_pool(name="wpool", bufs=int(os.environ.get("KERNEL_WBUFS", "3"))) as wpool,
        tc.tile_pool(name="hv", bufs=16) as hv,
        tc.tile_pool(name="scp", bufs=24) as scp,
        tc.tile_pool(name="ftp", bufs=1) as ftp,
        tc.tile_pool(name="z3p", bufs=2) as z3p,
        tc.tile_pool(name="sx4", bufs=5) as sx4,
        tc.tile_pool(name="e2", bufs=10) as e2,
        tc.tile_pool(name="stp", bufs=24) as stp,
        tc.tile_pool(name="psp", bufs=8, space="PSUM") as psp,
        tc.tile_pool(name="pdram", bufs=2, space="DRAM") as pdram,
    ):
        identb = constp.tile([P, P], bf16, name="identb")
        make_identity(nc, identb[:])
        # lower-triangular (incl. diagonal) 0/1 mask for the band edge
        tri = constp.tile([P, P], f32, name="tri")
        nc.gpsimd.memset(tri[:], 1.0)
        nc.gpsimd.affine_select(
            out=tri[:], in_=tri[:], compare_op=OP.is_ge, fill=0.0,
            base=0, pattern=[[-1, P]], channel_multiplier=1,
        )
        bf1_sb = constp.tile([P, CH_F], f32, name="bf1_sb")
        nc.sync.dma_start(out=bf1_sb[:], in_=tn["bf1p"].rearrange("(k p) -> p k", p=P))
        ones_sb = constp.tile([1, P], bf16, name="ones_sb")
        nc.sync.dma_start(out=ones_sb[:], in_=tn["ones_row"][:, :])
        bmr = constp.tile([1, D], bf16, name="bmr")
        nc.sync.dma_start(out=bmr[:], in_=tn["bmlp_row"][:, :])
        bf2r = constp.tile([1, D], bf16, name="bf2r")
        nc.sync.dma_start(out=bf2r[:], in_=tn["bf2_row"][:, :])
        u_ke = []
        for dc in range(CH_D):
            ut = constp.tile([P, C], bf16, name=f"uke{dc}")
            nc.sync.dma_start(out=ut[:], in_=tn["u_ke"][dc * P:(dc + 1) * P, :])
            u_ke.append(ut)
        if has_bke:
            wqbke = constp.tile([P, CH_D], bf16, name="wqbke")
            nc.sync.dma_start(
                out=wqbke[:], in_=tn["wqbke"].rearrange("(k p) -> p k", p=P))
            ubke = constp.tile([P, CH_C], f32, name="ubke")
            nc.sync.dma_start(
                out=ubke[:], in_=tn["ubke"].rearrange("(k p) -> p k", p=P))

        def _one_pass():
            for bi in range(BPC):
                # ---- hf_T [d, w] = (h | x)^T via xbar DMA; hf_nat plain ----
                hf = []
                for dc in range(CH_D):
                    t = hv.tile([P, W], bf16, name=f"hf{dc}", tag="hv")
                    nc.sync.dma_start(
                        out=t[:, 0:MEM], in_=hs[bi, :, dc * P:(dc + 1) * P],
                        transpose=True,
                    )
                    nc.sync.dma_start(
                        out=t[:, MEM:W], in_=xs[bi, :, dc * P:(dc + 1) * P],
                        transpose=True,
                    )
                    hf.append(t)
                hfn = []
                for wc in range(CH_W):
                    t = hv.tile([P, D], bf16, name=f"hfn{wc}", tag="hv")
                    src = hs if wc < CH_C else xs
                    ro = wc * P if wc < CH_C else (wc - CH_C) * P
                    nc.sync.dma_start(out=t[:], in_=src[bi, ro:ro + P, :])
                    hfn.append(t)

                # ---- S: sT = Wqke^T x^T + u_ke  [d_in, c] ----
                wqke = wpool.tile([P, CH_D, D], bf16, name="wqke", tag="w")
                nc.sync.dma_start(out=wqke[:], in_=tn["wqke_r"][:, :, :])
                sT = []
                for do in range(CH_D):
                    sps = psp.tile([P, C], f32, name="sps", tag="ps")
                    for ki in range(CH_D):
                        nc.tensor.matmul(
                            sps[:],
                            wqke[:, ki, do * P:(do + 1) * P],
                            hf[ki][:, MEM:W],
                            start=(ki == 0), stop=(ki == CH_D - 1),
                        )
                    st = scp.tile([P, C], bf16, name=f"sT{do}", tag="sc")
                    vtt(st[:], sps[:], u_ke[do][:], OP.add)
                    sT.append(st)
                dume = stp.tile([P, 1], f32, name="dume", tag="st")
                nc.scalar.activation(dume[:], sT[0][:, 0:1], AF.Exp)

                # ---- E: Pm = x @ krq^T -> per-row-block DRAM scratch (bf16);
                #      each 129-row tile is self-contained for its skew read ----
                krq = wpool.tile([P, CH_D, W], bf16, name="krq", tag="w")
                nc.sync.dma_start(out=krq[:], in_=tn["krq_r"][:, :, :])
                pd = pdram.tile([C * W], bf16, name="pd", tag="pd")
                pd2 = pd.rearrange("(c w) -> c w", w=W)
                for ci in range(CH_C):
                    pps = [psp.tile([P, C], f32, name=f"pps{hh}", tag="ps")
                           for hh in range(2)]
                    for ki in range(CH_D):
                        for hh in range(2):
                            nc.tensor.matmul(
                                pps[hh][:],
                                hf[ki][:, MEM + ci * P:MEM + (ci + 1) * P],
                                krq[:, ki, hh * 512:(hh + 1) * 512],
                                start=(ki == 0), stop=(ki == CH_D - 1),
                            )
                    for hh in range(2):
                        qt = e2.tile([P, C], bf16, name="qt", tag="e2")
                        nc.sync.dma_start(
                            out=qt[:],
                            in_=tn["qfold"][ci * P:(ci + 1) * P,
                                            hh * 512:(hh + 1) * 512],
                        )
                        psb = e2.tile([P, C], bf16, name="psb", tag="e2")
                        vtt(psb[:], pps[hh][:], qt[:], OP.add)
                        nc.sync.dma_start(
                            out=pd2[ci * P:(ci + 1) * P, hh * 512:(hh + 1) * 512],
                            in_=psb[:],
                        )

                # ---- delta row-offset (only when bke != 0) ----
                deltas = []
                if has_bke:
                    for ci in range(CH_C):
                        dps = psp.tile([P, 1], f32, name="dps", tag="ps")
                        for ki in range(CH_D):
                            nc.tensor.matmul(
                                dps[:],
                                hf[ki][:, MEM + ci * P:MEM + (ci + 1) * P],
                                wqbke[:, ki:ki + 1],
                                start=(ki == 0), stop=(ki == CH_D - 1),
                            )
                        dcol = stp.tile([P, 1], f32, name="dcol", tag="st")
                        vtt(dcol[:], dps[:], ubke[:, ci:ci + 1], OP.add)
                        deltas.append(dcol)

                # ---- F+O fused per row-block: logits, unnormalized softmax,
                #      transpose, att@hf; 1/z folded into the oh evacuation ----
                es, rzs = [], []
                for ci in range(CH_C):
                    active = ci * P + 640          # cols >= active fully masked
                    up = active - 512
                    pskew = e2.tile([P, W], bf16, name="pskew", tag="e2")
                    skew_ap = bass.AP(
                        tensor=pd.tensor,
                        offset=pd.offset + (W - 1) * P * ci + C - 1,
                        ap=[[W - 1, P], [1, W]],
                    )
                    nc.sync.dma_start(out=pskew[:], in_=skew_ap)
                    aps0 = psp.tile([P, C], f32, name="aps0", tag="ps")
                    aps1 = psp.tile([P, up], f32, name="aps1", tag="ps")
                    for ki in range(CH_D):
                        nc.tensor.matmul(
                            aps0[:],
                            sT[ki][:, ci * P:(ci + 1) * P],
                            hf[ki][:, 0:512],
                            start=(ki == 0), stop=(ki == CH_D - 1),
                        )
                        nc.tensor.matmul(
                            aps1[:],
                            sT[ki][:, ci * P:(ci + 1) * P],
                            hf[ki][:, 512:512 + up],
                            start=(ki == 0), stop=(ki == CH_D - 1),
                        )
                    t = sx4.tile([P, W], f32, name="t", tag="sx")
                    vtt(t[:, 0:512], aps0[:], pskew[:, 0:512], OP.add)
                    vtt(t[:, 512:active], aps1[:], pskew[:, 512:active], OP.add)
                    if has_bke:
                        nc.vector.tensor_scalar(
                            out=t[:, 0:active], in0=t[:, 0:active],
                            scalar1=deltas[ci][:], scalar2=None, op0=OP.add,
                        )
                    vtt(t[:, active - P:active], t[:, active - P:active],
                        tri[:], OP.mult)
                    e = e2.tile([P, W], bf16, name=f"e{ci}", tag="e2")
                    zrow = stp.tile([P, 1], f32, name="zrow", tag="st")
                    if active < W:
                        nc.vector.memset(e[:, active:W], 1.0)
                    nc.scalar.activation(e[:, 0:active], t[:, 0:active], AF.Exp,
                                         scale=ISQ, accum_out=zrow[:])
                    if active < W:
                        nc.vector.tensor_scalar_add(zrow[:], zrow[:], float(W - active))
                    rz = stp.tile([P, 1], f32, name="rz", tag="st")
                    nc.vector.reciprocal(rz[:], zrow[:])
                    rzs.append(rz)
                    es.append(e)
                attT = []
                for wc in range(CH_W):
                    tp = psp.tile([P, C], bf16, name="tp2", tag="ps")
                    for ci in range(CH_C):
                        nc.tensor.transpose(
                            tp[:, ci * P:(ci + 1) * P],
                            es[ci][:, wc * P:(wc + 1) * P],
                            identb[:],
                        )
                    at = scp.tile([P, C], bf16, name=f"attT{wc}", tag="sc")
                    evac(at[:], tp[:])
                    attT.append(at)

                # ---- O: oh = att @ hf  [c, d]; 1/z applied at evacuation ----
                ohps = [
                    [psp.tile([P, C], f32, name=f"oh{ci}{hh}", tag="ps") for hh in range(2)]
                    for ci in range(CH_C)
                ]
                for wc in range(CH_W):
                    for ci in range(CH_C):
                        for hh in range(2):
                            nc.tensor.matmul(
                                ohps[ci][hh][:],
                                attT[wc][:, ci * P:(ci + 1) * P],
                                hfn[wc][:, hh * 512:(hh + 1) * 512],
                                start=(wc == 0), stop=(wc == CH_W - 1),
                            )
                ohs = []
                for ci in range(CH_C):
                    oh = e2.tile([P, D], bf16, name="ohsb", tag="e2")
                    for hh in range(2):
                        alt_state[0] ^= 1
                        if alt_state[0]:
                            nc.scalar.mul(oh[:, hh * 512:(hh + 1) * 512],
                                          ohps[ci][hh][:], rzs[ci][:])
                        else:
                            nc.vector.tensor_scalar_mul(
                                oh[:, hh * 512:(hh + 1) * 512], ohps[ci][hh][:],
                                rzs[ci][:])
                    ohs.append(oh)
                oT = []
                for dc in range(CH_D):
                    tp = psp.tile([P, C], bf16, name="tp4", tag="ps")
                    for ci in range(CH_C):
                        nc.tensor.transpose(
                            tp[:, ci * P:(ci + 1) * P],
                            ohs[ci][:, dc * P:(dc + 1) * P],
                            identb[:],
                        )
                    ot = scp.tile([P, C], bf16, name=f"oT{dc}", tag="sc")
                    evac(ot[:], tp[:])
                    oT.append(ot)
                dums = stp.tile([P, 1], f32, name="dums", tag="st")
                nc.scalar.activation(dums[:], tri[:, 0:1], AF.Sqrt)

                # ---- H: o2 = oh @ Wvm^T + bmlp_adj ; LN -> z (bf16) ; zT ----
                wvm = wpool.tile([P, CH_D, D], bf16, name="wvm", tag="w")
                nc.sync.dma_start(out=wvm[:], in_=tn["wvm_r"][:, :, :])
                zs = []
                for ci in range(CH_C):
                    o2 = sx4.tile([P, D], f32, name="o2", tag="sx")
                    o2ps = [psp.tile([P, C], f32, name=f"o2ps{hh}", tag="ps")
                            for hh in range(2)]
                    for ki in range(CH_D):
                        for hh in range(2):
                            nc.tensor.matmul(
                                o2ps[hh][:],
                                oT[ki][:, ci * P:(ci + 1) * P],
                                wvm[:, ki, hh * 512:(hh + 1) * 512],
                                start=(ki == 0), stop=False,
                            )
                    shs, sss = [], []
                    for hh in range(2):
                        nc.tensor.matmul(
                            o2ps[hh][:], ones_sb[:1, :P], bmr[:1, hh * 512:(hh + 1) * 512],
                            start=False, stop=True,
                        )
                        shh = stp.tile([P, 1], f32, name="shh", tag="st")
                        nc.scalar.activation(
                            o2[:, hh * 512:(hh + 1) * 512], o2ps[hh][:], AF.Identity,
                            bias=0.0, accum_out=shh[:],
                        )
                        shs.append(shh)
                    sq = sx4.tile([P, D], f32, name="sq", tag="sx")
                    for hh in range(2):
                        ssh = stp.tile([P, 1], f32, name="ssh", tag="st")
                        nc.scalar.activation(
                            sq[:, hh * 512:(hh + 1) * 512],
                            o2[:, hh * 512:(hh + 1) * 512], AF.Square,
                            bias=0.0, accum_out=ssh[:],
                        )
                        sss.append(ssh)
                    sh = stp.tile([P, 1], f32, name="sh", tag="st")
                    vtt(sh[:], shs[0][:], shs[1][:], OP.add)
                    ss = stp.tile([P, 1], f32, name="ss", tag="st")
                    vtt(ss[:], sss[0][:], sss[1][:], OP.add)
                    mu = stp.tile([P, 1], f32, name="mu", tag="st")
                    nc.vector.tensor_scalar_mul(mu[:], sh[:], 1.0 / D)
                    t1 = stp.tile([P, 1], f32, name="t1", tag="st")
                    vtt(t1[:], sh[:], sh[:], OP.mult)
                    t2 = stp.tile([P, 1], f32, name="t2", tag="st")
                    nc.vector.tensor_scalar(
                        out=t2[:], in0=t1[:], scalar1=-1.0 / D, scalar2=ss[:],
                        op0=OP.mult, op1=OP.add,
                    )
                    t3 = stp.tile([P, 1], f32, name="t3", tag="st")
                    nc.vector.tensor_scalar(
                        out=t3[:], in0=t2[:], scalar1=1.0 / D, scalar2=EPS,
                        op0=OP.mult, op1=OP.add,
                    )
                    sd = stp.tile([P, 1], f32, name="sd", tag="st")
                    nc.scalar.activation(sd[:], t3[:], AF.Sqrt, bias=0.0)
                    rstd = stp.tile([P, 1], f32, name="rstd", tag="st")
                    nc.vector.reciprocal(rstd[:], sd[:])
                    zc = e2.tile([P, D], bf16, name="z", tag="e2")
                    nc.vector.tensor_scalar(
                        out=zc[:], in0=o2[:], scalar1=mu[:], scalar2=rstd[:],
                        op0=OP.subtract, op1=OP.mult,
                    )
                    zs.append(zc)
                zT3 = z3p.tile([P, CH_D, C], bf16, name="zT3", tag="z3")
                for dc in range(CH_D):
                    tp = psp.tile([P, C], bf16, name="tp3", tag="ps")
                    for ci in range(CH_C):
                        nc.tensor.transpose(
                            tp[:, ci * P:(ci + 1) * P],
                            zs[ci][:, dc * P:(dc + 1) * P],
                            identb[:],
                        )
                    evac(zT3[:, dc, :], tp[:])

                # ---- I: f_T = relu(Wf1g z_T + bf1') ----
                fT3 = ftp.tile([P, CH_F, C], bf16, name="fT3", tag="ft")
                for jj in range(4):
                    w1c = wpool.tile([P, CH_D, 1024], bf16, name=f"w1c{jj}", tag="w")
                    nc.sync.dma_start(
                        out=w1c[:], in_=tn["w1_r"][:, :, jj * 1024:(jj + 1) * 1024]
                    )
                    for j in range(8):
                        jc = jj * 8 + j
                        fps = psp.tile([P, C], f32, name="fps", tag="ps")
                        for dc in range(CH_D):
                            nc.tensor.matmul(
                                fps[:], w1c[:, dc, j * P:(j + 1) * P], zT3[:, dc, :],
                                start=(dc == 0), stop=(dc == CH_D - 1),
                            )
                        alt_state[0] ^= 1
                        if alt_state[0]:
                            nc.scalar.activation(fT3[:, jc, :], fps[:], AF.Relu,
                                                 bias=bf1_sb[:, jc:jc + 1])
                        else:
                            nc.vector.tensor_scalar(
                                out=fT3[:, jc, :], in0=fps[:],
                                scalar1=bf1_sb[:, jc:jc + 1],
                                scalar2=0.0, op0=OP.add, op1=OP.max,
                            )

                # ---- J: out = f @ Wf2^T + bf2 ----
                outps = [
                    [psp.tile([P, C], f32, name=f"op{ci}{hh}", tag="ps") for hh in range(2)]
                    for ci in range(CH_C)
                ]
                for jj in range(4):
                    w2c = wpool.tile([P, 8, D], bf16, name=f"w2c{jj}", tag="w")
                    nc.sync.dma_start(
                        out=w2c[:], in_=tn["w2_r"][:, jj * 8:(jj + 1) * 8, :]
                    )
                    for j in range(8):
                        jc = jj * 8 + j
                        for ci in range(CH_C):
                            for hh in range(2):
                                nc.tensor.matmul(
                                    outps[ci][hh][:],
                                    fT3[:, jc, ci * P:(ci + 1) * P],
                                    w2c[:, j, hh * 512:(hh + 1) * 512],
                                    start=(jc == 0), stop=False,
                                )
                for ci in range(CH_C):
                    ob = sx4.tile([P, D], f32, name="ob", tag="sx")
                    for hh in range(2):
                        nc.tensor.matmul(
                            outps[ci][hh][:], ones_sb[:1, :P],
                            bf2r[:1, hh * 512:(hh + 1) * 512],
                            start=False, stop=True,
                        )
                        evac(ob[:, hh * 512:(hh + 1) * 512], outps[ci][hh][:])
                    nc.sync.dma_start(out=outs[bi, ci * P:(ci + 1) * P, :], in_=ob[:])

        LOOP_R = int(os.environ.get("KERNEL_LOOP", "0"))
        if LOOP_R > 1:
            with tc.For_i(0, LOOP_R, 1):
                _one_pass()
        else:
            _one_pass()


def _build(has_bke=False):
    key = ("nc", has_bke)
    if key in _cached:
        return _cached[key]
    import concourse.mybir as mybir
    import concourse.tile as tile
    from concourse import bacc

    f32 = mybir.dt.float32
    bf16 = mybir.dt.bfloat16
    nc = bacc.Bacc("TRN2", target_bir_lowering=False, debug=False,
                   num_devices=NCORES)
    tn = {}
    tn["x"] = nc.dram_tensor("x", [BPC, C, D], bf16, kind="ExternalInput")
    tn["h"] = nc.dram_tensor("h", [BPC, MEM, D], bf16, kind="ExternalInput")
    for nm, shp in [
        ("wqke_r", [P, CH_D, D]), ("wvm_r", [P, CH_D, D]), ("krq_r", [P, CH_D, W]),
        ("w1_r", [P, CH_D, FF]), ("w2_r", [P, CH_F, D]),
        ("u_ke", [D, C]), ("qfold", [C, W]),
        ("ones_row", [1, P]), ("bmlp_row", [1, D]), ("bf2_row", [1, D]),
    ]:
        tn[nm] = nc.dram_tensor(nm, shp, bf16, kind="ExternalInput")
    for nm, shp in [("bf1p", [FF])]:
        tn[nm] = nc.dram_tensor(nm, shp, f32, kind="ExternalInput")
    if has_bke:
        tn["wqbke"] = nc.dram_tensor("wqbke", [D], bf16, kind="ExternalInput")
        tn["ubke"] = nc.dram_tensor("ubke", [C], f32, kind="ExternalInput")
    tn["out"] = nc.dram_tensor("out", [BPC, C, D], f32, kind="ExternalOutput")

    with tile.TileContext(nc) as tc:
        _emit(nc, tc, tn, has_bke)
    nc.compile()
    _cached[key] = nc
    return nc


def _circulant_idx(height, width, shift):
    i = np.roll(np.arange(width), shift)[::-1]
    i2 = np.concatenate([i, i])
    win = np.lib.stride_tricks.sliding_window_view(i2, width)
    win = win[:, ::-1]
    return np.ascontiguousarray(win[:height])


def _host_consts(inputs):
    f = np.float32
    bf = ml_dtypes.bfloat16
    Wq, bq = inputs["Wq"].astype(f), inputs["bq"].astype(f)
    Wke, bke = inputs["Wke"].astype(f), inputs["bke"].astype(f)
    Wkr, bkr = inputs["Wkr"].astype(f), inputs["bkr"].astype(f)
    Wv, bv = inputs["Wv"].astype(f), inputs["bv"].astype(f)
    Wmlp, bmlp = inputs["Wmlp"].astype(f), inputs["bmlp"].astype(f)
    gamma, beta = inputs["gamma"].astype(f), inputs["beta"].astype(f)
    Wf1, bf1 = inputs["Wf1"].astype(f), inputs["bf1"].astype(f)
    Wf2, bf2 = inputs["Wf2"].astype(f), inputs["bf2"].astype(f)
    u, v, rr = inputs["u"].astype(f), inputs["v"].astype(f), inputs["r"].astype(f)

    kr = rr @ Wkr.T + bkr                      # [W, D]
    u_b = u + bq[None, :]                      # bq rides along with u
    vkr = v @ kr.T                             # [C, W]
    ukr = u_b @ kr.T                           # [C, W]
    idx = _circulant_idx(C, W, -C + 1)
    skew_ukr = np.take_along_axis(ukr, idx, axis=1)
    vkadj = vkr - skew_ukr                     # unscaled; mask handled on-chip
    # inverse-skew vkadj into the scratch-matrix domain: adding qfold to Pm
    # before the skewed re-read reproduces "+ vkadj" at every in-band (c, w)
    qfold = np.zeros((C, W), np.float32)
    for c in range(C):
        qfold[c, C - 1 - c:W] = vkadj[c, 0:c + 513]

    Wqke = Wke.T @ Wq                          # s^T = Wqke^T x^T (+ (u Wke)^T)
    krq = kr @ Wq                              # [W, D]: x @ krq^T = (x Wq^T) kr^T
    Wvm = Wmlp @ Wv                            # o2 = oh @ Wvm^T

    def pkd(Wt, width):                        # [D, width] -> [P, CH_D, width]
        return np.ascontiguousarray(
            Wt.reshape(CH_D, P, width).transpose(1, 0, 2)
        ).astype(bf)

    cn = {
        "wqke_r": pkd(np.ascontiguousarray(Wqke.T), D),
        "wvm_r": pkd(np.ascontiguousarray(Wvm.T), D),
        "krq_r": pkd(np.ascontiguousarray(krq.T), W),
        "w1_r": pkd(np.ascontiguousarray((Wf1 * gamma[None, :]).T), FF),
        "w2_r": np.ascontiguousarray(
            Wf2.T.reshape(CH_F, P, D).transpose(1, 0, 2)
        ).astype(bf),
        "u_ke": np.ascontiguousarray((u_b @ Wke).T).astype(bf),
        "qfold": qfold.astype(bf),
        "bf1p": (bf1 + Wf1 @ beta).astype(f),
        "ones_row": np.ones((1, P), bf),
        "bmlp_row": (bmlp + Wmlp @ bv).reshape(1, D).astype(bf),
        "bf2_row": bf2.reshape(1, D).astype(bf),
    }
    has_bke = bool(np.any(bke != 0.0))
    if has_bke:
        cn["wqbke"] = (Wq.T @ bke).astype(bf)
        cn["ubke"] = (u_b @ bke).astype(f)
    return cn, has_bke


def kernel(**inputs):
    from concourse.bass_utils import run_bass_kernel_spmd

    cn, has_bke = _host_consts(inputs)
    nc = _build(has_bke)
    bf = ml_dtypes.bfloat16
    x = np.ascontiguousarray(inputs["x"]).astype(bf)
    h = np.ascontiguousarray(inputs["h"]).astype(bf)
    in_maps = []
    for i in range(NCORES):
        m = dict(cn)
        m["x"] = np.ascontiguousarray(x[i * BPC:(i + 1) * BPC])
        m["h"] = np.ascontiguousarray(h[i * BPC:(i + 1) * BPC])
        in_maps.append(m)
    res = run_bass_kernel_spmd(nc, in_maps, list(range(NCORES)))
    out = np.concatenate([res.results[i]["out"] for i in range(NCORES)], axis=0)
    return out.astype(np.float32)
